# revision 1
# baseline (speedup 1.0000x reference)
"""LoRA-MLP kernel for 8x TRN2 NeuronCores (SPMD data-parallel over batch).

Math (per batch b):
    z1 = (x @ v) / IN            [F, R]
    z  = (z1 @ u.T) / R          [F, OUT]
    y  = gelu(x @ W.T + fc_bias + z + b)

Device formulation (per core, 4 batches), all PSUM-accumulated per f-tile:
    psum[f, o] = ones[1,f].T @ bias[1,o]          (K=1: fc_bias + b)
               + sum_k xT[k][:, f].T @ WT[k][:, o]  (8 K-tiles of 128)
               + z1T[:, f].T @ uT[:, o]             (K=16 LoRA)
    out = gelu(psum)   (ScalarE, PSUM -> SBUF fp32)
    z1T[r, f] = sum_k vs[k].T @ xT[k]  on PE, copied PSUM->SBUF via ScalarE.

All matmul operands bf16 (host-cast/laid out); fp32 accumulation in PSUM.
Sync-wait budget note: this codegen allows roughly one semaphore wait per
compute instruction (2 for DMA), so pools are sized for zero slot reuse and
each producer/consumer pair crosses engines exactly once.
"""

import sys

for _p in ("/opt/trn_rl_repo", "/opt/pypackages"):
    if _p not in sys.path:
        sys.path.append(_p)

import numpy as np
import ml_dtypes

B, F, IN, OUT, R = 32, 512, 1024, 1024, 16
NCORES = 8
BPC = B // NCORES  # batches per core = 4
KT = IN // 128  # 8 K-tiles
FT = F // 128  # 4 F-tiles per batch
BF16 = ml_dtypes.bfloat16

_COMPILED = {}


def _build_nc():
    import concourse.tile as tile
    from concourse import bacc, mybir

    # Bacc (not raw Bass): its compile() runs generate_event_semaphores,
    # which splits multi-sem waits — walrus codegen allows only one sync
    # wait per instruction.
    nc = bacc.Bacc(None)
    bf = mybir.dt.bfloat16
    f32 = mybir.dt.float32

    xt = nc.declare_dram_parameter("xt", [BPC, 128, KT, F], bf, isOutput=False)
    wt = nc.declare_dram_parameter("wt", [128, KT, OUT], bf, isOutput=False)
    vs = nc.declare_dram_parameter("vs", [BPC, 128, KT, R], bf, isOutput=False)
    ut = nc.declare_dram_parameter("ut", [BPC, R, OUT], bf, isOutput=False)
    bias = nc.declare_dram_parameter("bias", [BPC, 1, OUT], bf, isOutput=False)
    ones = nc.declare_dram_parameter("ones", [1, 128], bf, isOutput=False)
    y = nc.declare_dram_parameter("y", [BPC, FT, 128, OUT], f32, isOutput=True)

    GELU = mybir.ActivationFunctionType.Gelu

    with tile.TileContext(nc) as tc:
        with (
            tc.tile_pool(name="const", bufs=1) as const_pool,
            tc.tile_pool(name="xin", bufs=BPC) as xin_pool,
            tc.tile_pool(name="small", bufs=BPC) as small_pool,
            tc.tile_pool(name="out", bufs=FT * BPC) as out_pool,
            tc.tile_pool(name="psum", bufs=6, space="PSUM") as psum_pool,
            tc.tile_pool(name="zpsum", bufs=2, space="PSUM") as zpsum_pool,
        ):
            wt_sb = const_pool.tile([128, KT, OUT], bf)
            nc.sync.dma_start(out=wt_sb[:], in_=wt[:])
            ones_sb = const_pool.tile([1, 128], bf)
            nc.sync.dma_start(out=ones_sb[:], in_=ones[:])

            z1_tiles = [
                const_pool.tile([R, F], bf, name=f"z1_{i}", tag=f"z1_{i}")
                for i in range(BPC)
            ]

            for b in range(BPC):
                xt_sb = xin_pool.tile([128, KT, F], bf, tag="xt")
                nc.sync.dma_start(out=xt_sb[:], in_=xt[b])
                vs_sb = small_pool.tile([128, KT, R], bf, tag="vs")
                nc.sync.dma_start(out=vs_sb[:], in_=vs[b])
                ut_sb = small_pool.tile([R, OUT], bf, tag="ut")
                nc.sync.dma_start(out=ut_sb[:], in_=ut[b])
                bias_sb = small_pool.tile([1, OUT], bf, tag="bias")
                nc.sync.dma_start(out=bias_sb[:], in_=bias[b])

                # Stage 1: z1T[r, f] = sum_k vs[k].T @ xT[k]  -> [16, F] PSUM
                z1_ps = zpsum_pool.tile([R, F], f32, tag="z1ps")
                for k in range(KT):
                    nc.tensor.matmul(
                        z1_ps[:],
                        lhsT=vs_sb[:, k, :],
                        rhs=xt_sb[:, k, :],
                        start=(k == 0),
                        stop=(k == KT - 1),
                    )
                z1_sb = z1_tiles[b]
                nc.scalar.copy(z1_sb[:], z1_ps[:])

                # Stage 2: bias + main matmul + LoRA, accumulated in PSUM.
                for ft in range(FT):
                    fsl = slice(ft * 128, (ft + 1) * 128)
                    ps0 = psum_pool.tile([128, 512], f32, tag="ps")
                    ps1 = psum_pool.tile([128, 512], f32, tag="ps")
                    nc.tensor.matmul(
                        ps0[:], lhsT=ones_sb[:], rhs=bias_sb[:, 0:512],
                        start=True, stop=False,
                    )
                    nc.tensor.matmul(
                        ps1[:], lhsT=ones_sb[:], rhs=bias_sb[:, 512:1024],
                        start=True, stop=False,
                    )
                    for k in range(KT):
                        lhsT = xt_sb[:, k, fsl]
                        nc.tensor.matmul(
                            ps0[:], lhsT=lhsT, rhs=wt_sb[:, k, 0:512],
                            start=False, stop=False,
                        )
                        nc.tensor.matmul(
                            ps1[:], lhsT=lhsT, rhs=wt_sb[:, k, 512:1024],
                            start=False, stop=False,
                        )
                    nc.tensor.matmul(
                        ps0[:], lhsT=z1_sb[:, fsl], rhs=ut_sb[:, 0:512],
                        start=False, stop=True,
                    )
                    nc.tensor.matmul(
                        ps1[:], lhsT=z1_sb[:, fsl], rhs=ut_sb[:, 512:1024],
                        start=False, stop=True,
                    )
                    # One [128, 1024] tile per f-tile: both gelu halves land in
                    # it, then a single 512KB store (4KB/partition lines).
                    # Bacc's generate_event_semaphores legalizes the DMA's two
                    # ACT waits.
                    o01 = out_pool.tile([128, OUT], f32, tag="o")
                    nc.scalar.activation(o01[:, 0:512], ps0[:], GELU)
                    nc.scalar.activation(o01[:, 512:1024], ps1[:], GELU)
                    nc.sync.dma_start(out=y[b, ft], in_=o01[:])
    nc.finalize()
    return nc


def _shard_inputs(x, u, v, b, W, fc_bias):
    """Build per-core device input dicts (host-side layout + bf16 cast)."""
    # xt[c][bb, p, k, f] = x[4c+bb, f, 128k+p]
    xt = np.ascontiguousarray(
        x.reshape(B, F, KT, 128).transpose(0, 3, 2, 1)
    ).astype(BF16)
    # wt[p, k, o] = W[o, 128k+p]
    wt = np.ascontiguousarray(W.reshape(OUT, KT, 128).transpose(2, 1, 0)).astype(BF16)
    # vs[bb, p, k, r] = v[bb, 0, 128k+p, r] / (IN*R)
    vs = np.ascontiguousarray(
        (v[:, 0] / float(IN * R)).reshape(B, KT, 128, R).transpose(0, 2, 1, 3)
    ).astype(BF16)
    # ut[bb, r, o] = u[bb, 0, o, r]
    ut = np.ascontiguousarray(u[:, 0].transpose(0, 2, 1)).astype(BF16)
    bias = (fc_bias[None, None, :] + b).astype(BF16)  # [B, 1, OUT]

    in_maps = []
    for c in range(NCORES):
        s = slice(c * BPC, (c + 1) * BPC)
        in_maps.append(
            {
                "xt": xt[s],
                "wt": wt,
                "vs": vs[s],
                "ut": ut[s],
                "bias": np.ascontiguousarray(bias[s]),
                "ones": np.ones((1, 128), dtype=BF16),
            }
        )
    return in_maps


def _run(in_maps, trace=False, **kw):
    from concourse import bass_utils

    key = "nc"
    if key not in _COMPILED:
        _COMPILED[key] = _build_nc()
    nc = _COMPILED[key]
    res = bass_utils.run_bass_kernel_spmd(
        nc, in_maps, list(range(NCORES)), trace=trace, **kw
    )
    return res


def kernel(x, u, v, b, W, fc_bias):
    x = np.asarray(x, dtype=np.float32)
    u = np.asarray(u, dtype=np.float32)
    v = np.asarray(v, dtype=np.float32)
    b = np.asarray(b, dtype=np.float32)
    W = np.asarray(W, dtype=np.float32)
    fc_bias = np.asarray(fc_bias, dtype=np.float32)

    in_maps = _shard_inputs(x, u, v, b, W, fc_bias)
    res = _run(in_maps, trace=False)
    outs = [r["y"].reshape(BPC, F, OUT) for r in res.results]
    return np.concatenate(outs, axis=0).astype(np.float32)



# revision 3
# speedup vs baseline: 995.6305x; 995.6305x over previous
"""LoRA-MLP kernel for 8x TRN NeuronCores (SPMD data-parallel over batch).

Math (per batch b):
    z1 = (x @ v) / IN            [F, R]
    z  = (z1 @ u.T) / R          [F, OUT]
    y  = gelu(x @ W.T + fc_bias + z + b)

Device formulation (per core, 4 batches), all PSUM-accumulated per f-tile:
    psum[f, o] = sum_k xT[k][:, f].T @ WT[k][:, o]   (8 K-tiles of 128)
               + z1T'[:, f].T @ utb[:, o]            (K=17 LoRA; row 16 of
                 z1T' is constant 1.0 and row 16 of utb is fc_bias + b, so
                 the bias add rides along with the LoRA contraction)
    out = gelu(psum)   (ScalarE, PSUM -> SBUF bf16)
    z1T[r, f] = sum_k vs[k].T @ xT[k]  on PE, copied PSUM->SBUF via ScalarE.

All matmul operands bf16 (host-cast/laid out); fp32 accumulation in PSUM;
y stored bf16 and widened to fp32 on the host (adds ~1e-3 rel err, well
inside the 2e-2 gate, and halves the store traffic).

Execution path: a cached jit(shard_map(bass_exec)) executable per module
(built once per process), replacing per-call run_bass_kernel_spmd which
re-traced, re-jitted and re-transferred ~180MB host<->device per call.
Inputs can stay device-resident between executes (no donation -- the kernel
writes every element of y, so outputs need no zero-init aliasing).

`loop=L` unrolls the whole per-core program L times inside one NEFF so the
harness can measure per-iteration device time by slope (t(L2)-t(L1))/(L2-L1),
cancelling RPC/dispatch constants that dominate single-execute wall time.
"""

import sys

for _p in ("/opt/trn_rl_repo", "/opt/pypackages"):
    if _p not in sys.path:
        sys.path.append(_p)

import numpy as np
import ml_dtypes

B, F, IN, OUT, R = 32, 512, 1024, 1024, 16
NCORES = 8
BPC = B // NCORES  # batches per core = 4
KT = IN // 128  # 8 K-tiles
FT = F // 128  # 4 F-tiles per batch
BF16 = ml_dtypes.bfloat16

_RUNNERS = {}


def _build_nc(loop=1):
    import concourse.tile as tile
    from concourse import bacc, mybir

    # Bacc (not raw Bass): its compile() runs generate_event_semaphores,
    # which splits multi-sem waits -- walrus codegen allows only one sync
    # wait per instruction.
    nc = bacc.Bacc(None)
    bf = mybir.dt.bfloat16
    f32 = mybir.dt.float32

    xt = nc.declare_dram_parameter("xt", [BPC, 128, KT, F], bf, isOutput=False)
    wt = nc.declare_dram_parameter("wt", [128, KT, OUT], bf, isOutput=False)
    vs = nc.declare_dram_parameter("vs", [BPC, 128, KT, R], bf, isOutput=False)
    utb = nc.declare_dram_parameter("utb", [BPC, R + 1, OUT], bf, isOutput=False)
    y = nc.declare_dram_parameter("y", [BPC, FT, 128, OUT], bf, isOutput=True)

    GELU = mybir.ActivationFunctionType.Gelu

    with tile.TileContext(nc) as tc:
        with (
            tc.tile_pool(name="const", bufs=1) as const_pool,
            tc.tile_pool(name="xin", bufs=BPC) as xin_pool,
            tc.tile_pool(name="small", bufs=BPC) as small_pool,
            tc.tile_pool(name="out", bufs=FT * BPC) as out_pool,
            tc.tile_pool(name="psum", bufs=6, space="PSUM") as psum_pool,
            tc.tile_pool(name="zpsum", bufs=2, space="PSUM") as zpsum_pool,
        ):
            wt_sb = const_pool.tile([128, KT, OUT], bf)
            nc.sync.dma_start(out=wt_sb[:], in_=wt[:])

            # z1' tiles: rows 0..15 = LoRA z1 (rewritten per batch), row 16
            # = constant 1.0 (written once) so the K=17 LoRA matmul also
            # adds utb row 16 (= fc_bias + b).
            z1_tiles = [
                const_pool.tile([R + 1, F], bf, name=f"z1_{i}", tag=f"z1_{i}")
                for i in range(BPC)
            ]
            # Whole-tile memset (a partition-16 start is illegal for engine
            # ops); the per-batch PSUM copy rewrites rows 0..15, so row 16
            # stays 1.0.
            for i in range(BPC):
                nc.vector.memset(z1_tiles[i][:, :], 1.0)

            for _it in range(loop):
                for b in range(BPC):
                    xt_sb = xin_pool.tile([128, KT, F], bf, tag="xt")
                    nc.sync.dma_start(out=xt_sb[:], in_=xt[b])
                    vs_sb = small_pool.tile([128, KT, R], bf, tag="vs")
                    nc.sync.dma_start(out=vs_sb[:], in_=vs[b])
                    utb_sb = small_pool.tile([R + 1, OUT], bf, tag="utb")
                    nc.sync.dma_start(out=utb_sb[:], in_=utb[b])

                    # Stage 1: z1T[r, f] = sum_k vs[k].T @ xT[k] -> [16, F]
                    z1_ps = zpsum_pool.tile([R, F], f32, tag="z1ps")
                    for k in range(KT):
                        nc.tensor.matmul(
                            z1_ps[:],
                            lhsT=vs_sb[:, k, :],
                            rhs=xt_sb[:, k, :],
                            start=(k == 0),
                            stop=(k == KT - 1),
                        )
                    z1_sb = z1_tiles[b]
                    nc.scalar.copy(z1_sb[0:R, :], z1_ps[:])

                    # Stage 2: main matmul + (LoRA + bias), PSUM-accumulated.
                    for ft in range(FT):
                        fsl = slice(ft * 128, (ft + 1) * 128)
                        ps0 = psum_pool.tile([128, 512], f32, tag="ps")
                        ps1 = psum_pool.tile([128, 512], f32, tag="ps")
                        for k in range(KT):
                            lhsT = xt_sb[:, k, fsl]
                            nc.tensor.matmul(
                                ps0[:], lhsT=lhsT, rhs=wt_sb[:, k, 0:512],
                                start=(k == 0), stop=False,
                            )
                            nc.tensor.matmul(
                                ps1[:], lhsT=lhsT, rhs=wt_sb[:, k, 512:1024],
                                start=(k == 0), stop=False,
                            )
                        nc.tensor.matmul(
                            ps0[:], lhsT=z1_sb[:, fsl], rhs=utb_sb[:, 0:512],
                            start=False, stop=True,
                        )
                        nc.tensor.matmul(
                            ps1[:], lhsT=z1_sb[:, fsl], rhs=utb_sb[:, 512:1024],
                            start=False, stop=True,
                        )
                        # One [128, 1024] bf16 tile per f-tile: both gelu
                        # halves land in it, then a single 256KB store.
                        o01 = out_pool.tile([128, OUT], bf, tag="o")
                        nc.scalar.activation(o01[:, 0:512], ps0[:], GELU)
                        nc.scalar.activation(o01[:, 512:1024], ps1[:], GELU)
                        nc.sync.dma_start(out=y[b, ft], in_=o01[:])
    nc.finalize()
    return nc


def _get_runner(loop=1):
    """Build (once per process) the Bass module + jitted SPMD executable."""
    if loop in _RUNNERS:
        return _RUNNERS[loop]
    import jax
    from jax.experimental.shard_map import shard_map
    from jax.sharding import Mesh, NamedSharding, PartitionSpec

    from concourse import mybir
    from concourse.bass2jax import (
        _bass_exec_p,
        install_neuronx_cc_hook,
        partition_id_tensor,
    )

    install_neuronx_cc_hook()
    nc = _build_nc(loop)

    partition_name = nc.partition_id_tensor.name if nc.partition_id_tensor else None
    in_names, out_names, out_avals = [], [], []
    for alloc in nc.m.functions[0].allocations:
        if not isinstance(alloc, mybir.MemoryLocationSet):
            continue
        name = alloc.memorylocations[0].name
        if alloc.kind == "ExternalInput":
            if name != partition_name:
                in_names.append(name)
        elif alloc.kind == "ExternalOutput":
            out_names.append(name)
            out_avals.append(
                jax.core.ShapedArray(
                    tuple(alloc.tensor_shape), mybir.dt.np(alloc.dtype)
                )
            )
    n_params = len(in_names)
    n_outs = len(out_names)
    # PJRT custom-call results need operand buffers declared at the JAX
    # level; the zero arrays double as (donatable) output storage upstream.
    # We do NOT donate: the kernel writes every element of y, so the zeros
    # are never read and can stay device-resident across executes.
    all_in = list(in_names) + list(out_names)
    if partition_name is not None:
        all_in.append(partition_name)

    def _body(*args):
        operands = list(args)
        if partition_name is not None:
            operands.append(partition_id_tensor())
        outs = _bass_exec_p.bind(
            *operands,
            out_avals=tuple(out_avals),
            in_names=tuple(all_in),
            out_names=tuple(out_names),
            lowering_input_output_aliases=(),
            sim_require_finite=True,
            sim_require_nnan=True,
            nc=nc,
        )
        return tuple(outs)

    devices = jax.devices()[:NCORES]
    assert len(devices) == NCORES, f"need {NCORES} cores, got {len(jax.devices())}"
    mesh = Mesh(np.asarray(devices), ("core",))
    in_specs = (PartitionSpec("core"),) * (n_params + n_outs)
    out_specs = (PartitionSpec("core"),) * n_outs
    fn = jax.jit(
        shard_map(
            _body, mesh=mesh, in_specs=in_specs, out_specs=out_specs,
            check_rep=False,
        ),
        keep_unused=True,
    )
    r = {
        "fn": fn,
        "nc": nc,
        "in_names": in_names,
        "out_names": out_names,
        "out_avals": out_avals,
        "sharding": NamedSharding(mesh, PartitionSpec("core")),
    }
    _RUNNERS[loop] = r
    return r


def _shard_inputs(x, u, v, b, W, fc_bias):
    """Build per-core device input dicts (host-side layout + bf16 cast)."""
    # xt[c][bb, p, k, f] = x[4c+bb, f, 128k+p]
    xt = np.ascontiguousarray(
        x.reshape(B, F, KT, 128).transpose(0, 3, 2, 1)
    ).astype(BF16)
    # wt[p, k, o] = W[o, 128k+p]
    wt = np.ascontiguousarray(W.reshape(OUT, KT, 128).transpose(2, 1, 0)).astype(BF16)
    # vs[bb, p, k, r] = v[bb, 0, 128k+p, r] / (IN*R)
    vs = np.ascontiguousarray(
        (v[:, 0] / float(IN * R)).reshape(B, KT, 128, R).transpose(0, 2, 1, 3)
    ).astype(BF16)
    # utb rows 0..15: u.T; row 16: fc_bias + b (the z1' ones row picks it up)
    ut = u[:, 0].transpose(0, 2, 1)  # [B, R, OUT]
    bias = fc_bias[None, None, :] + b  # [B, 1, OUT]
    utb = np.ascontiguousarray(np.concatenate([ut, bias], axis=1)).astype(BF16)

    in_maps = []
    for c in range(NCORES):
        s = slice(c * BPC, (c + 1) * BPC)
        in_maps.append({"xt": xt[s], "wt": wt, "vs": vs[s], "utb": utb[s]})
    return in_maps


def _device_args(runner, in_maps):
    """Concat per-core inputs to global arrays and place them on device.

    Returns device-resident jax.Arrays (inputs + reusable zero buffers for
    the output operands) that can be passed to runner['fn'] repeatedly with
    no further host<->device traffic.
    """
    import jax

    nc = runner["nc"]
    if getattr(nc, "dbg_addr", None) is not None:
        zdbg = np.zeros((1, 2), np.uint32)
        in_maps = [{**m, nc.dbg_addr.name: zdbg} for m in in_maps]
    concat = [
        np.concatenate([np.asarray(m[name]) for m in in_maps], axis=0)
        for name in runner["in_names"]
    ]
    zeros = [
        np.zeros((NCORES * a.shape[0], *a.shape[1:]), a.dtype)
        for a in runner["out_avals"]
    ]
    return [jax.device_put(a, runner["sharding"]) for a in concat + zeros]


def kernel(x, u, v, b, W, fc_bias):
    x = np.asarray(x, dtype=np.float32)
    u = np.asarray(u, dtype=np.float32)
    v = np.asarray(v, dtype=np.float32)
    b = np.asarray(b, dtype=np.float32)
    W = np.asarray(W, dtype=np.float32)
    fc_bias = np.asarray(fc_bias, dtype=np.float32)

    in_maps = _shard_inputs(x, u, v, b, W, fc_bias)
    runner = _get_runner(loop=1)
    dev = _device_args(runner, in_maps)
    outs = runner["fn"](*dev)
    yi = runner["out_names"].index("y")
    yb = np.asarray(outs[yi])  # [B, FT, 128, OUT] bf16 global
    return np.ascontiguousarray(yb.astype(np.float32).reshape(B, F, OUT))


# revision 5
# speedup vs baseline: 36289.6164x; 36.4489x over previous
"""LoRA-MLP kernel for 8x TRN NeuronCores (SPMD data-parallel over batch).

Math (per batch b):
    z1 = (x @ v) / IN            [F, R]
    z  = (z1 @ u.T) / R          [F, OUT]
    y  = gelu(x @ W.T + fc_bias + z + b)

Device formulation (per core, 4 batches), all PSUM-accumulated per f-tile:
    psum[f, o] = sum_k xT[k][:, f].T @ WT[k][:, o]   (8 K-tiles of 128)
               + z1T'[:, f].T @ utb[:, o]            (K=17 LoRA; row 16 of
                 z1T' is constant 1.0 and row 16 of utb is fc_bias + b, so
                 the bias add rides along with the LoRA contraction)
    out = gelu(psum)   (ScalarE, PSUM -> SBUF bf16)
    z1T[r, f] = sum_k vs[k].T @ xT[k]  on PE, copied PSUM->SBUF via ScalarE.

All matmul operands bf16 (host-cast/laid out); fp32 accumulation in PSUM;
y stored bf16 and widened to fp32 on the host (adds ~1e-3 rel err, well
inside the 2e-2 gate, and halves the store traffic).

Execution path: a cached jit(shard_map(bass_exec)) executable per module
(built once per process), replacing per-call run_bass_kernel_spmd which
re-traced, re-jitted and re-transferred ~180MB host<->device per call.
Inputs can stay device-resident between executes (no donation -- the kernel
writes every element of y, so outputs need no zero-init aliasing).

`loop=L` unrolls the whole per-core program L times inside one NEFF so the
harness can measure per-iteration device time by slope (t(L2)-t(L1))/(L2-L1),
cancelling RPC/dispatch constants that dominate single-execute wall time.
"""

import sys

for _p in ("/opt/trn_rl_repo", "/opt/pypackages"):
    if _p not in sys.path:
        sys.path.append(_p)

import numpy as np
import ml_dtypes

B, F, IN, OUT, R = 32, 512, 1024, 1024, 16
NCORES = 8
BPC = B // NCORES  # batches per core = 4
KT = IN // 128  # 8 K-tiles
FT = F // 128  # 4 F-tiles per batch
BF16 = ml_dtypes.bfloat16

_RUNNERS = {}


def _build_nc(loop=1):
    import concourse.tile as tile
    from concourse import bacc, mybir

    # Bacc (not raw Bass): its compile() runs generate_event_semaphores,
    # which splits multi-sem waits -- walrus codegen allows only one sync
    # wait per instruction.
    nc = bacc.Bacc(None)
    bf = mybir.dt.bfloat16
    f32 = mybir.dt.float32

    xt = nc.declare_dram_parameter("xt", [BPC, 128, KT, F], bf, isOutput=False)
    wt = nc.declare_dram_parameter("wt", [128, KT, OUT], bf, isOutput=False)
    vs = nc.declare_dram_parameter("vs", [BPC, 128, KT, R], bf, isOutput=False)
    utb = nc.declare_dram_parameter("utb", [BPC, R + 1, OUT], bf, isOutput=False)
    # Leading `loop` dim: each unrolled iteration writes its own slice, so
    # the backend cannot dead-code-eliminate iterations 0..loop-2 (it does,
    # if they all target the same region -- measured t(L=9) == t(L=1)).
    y = nc.declare_dram_parameter("y", [loop, BPC, FT, 128, OUT], bf, isOutput=True)

    GELU = mybir.ActivationFunctionType.Gelu

    with tile.TileContext(nc) as tc:
        with (
            tc.tile_pool(name="const", bufs=1) as const_pool,
            tc.tile_pool(name="xin", bufs=BPC) as xin_pool,
            tc.tile_pool(name="small", bufs=BPC) as small_pool,
            tc.tile_pool(name="out", bufs=FT * BPC) as out_pool,
            tc.tile_pool(name="psum", bufs=6, space="PSUM") as psum_pool,
            tc.tile_pool(name="zpsum", bufs=2, space="PSUM") as zpsum_pool,
        ):
            wt_sb = const_pool.tile([128, KT, OUT], bf)
            nc.sync.dma_start(out=wt_sb[:], in_=wt[:])

            # z1' tiles: rows 0..15 = LoRA z1 (rewritten per batch), row 16
            # = constant 1.0 (written once) so the K=17 LoRA matmul also
            # adds utb row 16 (= fc_bias + b).
            z1_tiles = [
                const_pool.tile([R + 1, F], bf, name=f"z1_{i}", tag=f"z1_{i}")
                for i in range(BPC)
            ]
            # Whole-tile memset (a partition-16 start is illegal for engine
            # ops); the per-batch PSUM copy rewrites rows 0..15, so row 16
            # stays 1.0.
            for i in range(BPC):
                nc.vector.memset(z1_tiles[i][:, :], 1.0)

            for _it in range(loop):
                for b in range(BPC):
                    xt_sb = xin_pool.tile([128, KT, F], bf, tag="xt")
                    nc.sync.dma_start(out=xt_sb[:], in_=xt[b])
                    vs_sb = small_pool.tile([128, KT, R], bf, tag="vs")
                    nc.sync.dma_start(out=vs_sb[:], in_=vs[b])
                    utb_sb = small_pool.tile([R + 1, OUT], bf, tag="utb")
                    nc.sync.dma_start(out=utb_sb[:], in_=utb[b])

                    # Stage 1: z1T[r, f] = sum_k vs[k].T @ xT[k] -> [16, F]
                    z1_ps = zpsum_pool.tile([R, F], f32, tag="z1ps")
                    for k in range(KT):
                        nc.tensor.matmul(
                            z1_ps[:],
                            lhsT=vs_sb[:, k, :],
                            rhs=xt_sb[:, k, :],
                            start=(k == 0),
                            stop=(k == KT - 1),
                        )
                    z1_sb = z1_tiles[b]
                    nc.scalar.copy(z1_sb[0:R, :], z1_ps[:])

                    # Stage 2: main matmul + (LoRA + bias), PSUM-accumulated.
                    for ft in range(FT):
                        fsl = slice(ft * 128, (ft + 1) * 128)
                        ps0 = psum_pool.tile([128, 512], f32, tag="ps")
                        ps1 = psum_pool.tile([128, 512], f32, tag="ps")
                        for k in range(KT):
                            lhsT = xt_sb[:, k, fsl]
                            nc.tensor.matmul(
                                ps0[:], lhsT=lhsT, rhs=wt_sb[:, k, 0:512],
                                start=(k == 0), stop=False,
                            )
                            nc.tensor.matmul(
                                ps1[:], lhsT=lhsT, rhs=wt_sb[:, k, 512:1024],
                                start=(k == 0), stop=False,
                            )
                        nc.tensor.matmul(
                            ps0[:], lhsT=z1_sb[:, fsl], rhs=utb_sb[:, 0:512],
                            start=False, stop=True,
                        )
                        nc.tensor.matmul(
                            ps1[:], lhsT=z1_sb[:, fsl], rhs=utb_sb[:, 512:1024],
                            start=False, stop=True,
                        )
                        # One [128, 1024] bf16 tile per f-tile: both gelu
                        # halves land in it, then a single 256KB store.
                        o01 = out_pool.tile([128, OUT], bf, tag="o")
                        nc.scalar.activation(o01[:, 0:512], ps0[:], GELU)
                        nc.scalar.activation(o01[:, 512:1024], ps1[:], GELU)
                        nc.sync.dma_start(out=y[_it, b, ft], in_=o01[:])
    nc.finalize()
    return nc


def _get_runner(loop=1):
    """Build (once per process) the Bass module + jitted SPMD executable."""
    if loop in _RUNNERS:
        return _RUNNERS[loop]
    import jax
    from jax.experimental.shard_map import shard_map
    from jax.sharding import Mesh, NamedSharding, PartitionSpec

    from concourse import mybir
    from concourse.bass2jax import (
        _bass_exec_p,
        install_neuronx_cc_hook,
        partition_id_tensor,
    )

    install_neuronx_cc_hook()
    nc = _build_nc(loop)

    partition_name = nc.partition_id_tensor.name if nc.partition_id_tensor else None
    in_names, out_names, out_avals = [], [], []
    for alloc in nc.m.functions[0].allocations:
        if not isinstance(alloc, mybir.MemoryLocationSet):
            continue
        name = alloc.memorylocations[0].name
        if alloc.kind == "ExternalInput":
            if name != partition_name:
                in_names.append(name)
        elif alloc.kind == "ExternalOutput":
            out_names.append(name)
            out_avals.append(
                jax.core.ShapedArray(
                    tuple(alloc.tensor_shape), mybir.dt.np(alloc.dtype)
                )
            )
    n_params = len(in_names)
    n_outs = len(out_names)
    # PJRT custom-call results need operand buffers declared at the JAX
    # level; the zero arrays double as (donatable) output storage upstream.
    # We do NOT donate: the kernel writes every element of y, so the zeros
    # are never read and can stay device-resident across executes.
    all_in = list(in_names) + list(out_names)
    if partition_name is not None:
        all_in.append(partition_name)

    def _body(*args):
        operands = list(args)
        if partition_name is not None:
            operands.append(partition_id_tensor())
        outs = _bass_exec_p.bind(
            *operands,
            out_avals=tuple(out_avals),
            in_names=tuple(all_in),
            out_names=tuple(out_names),
            lowering_input_output_aliases=(),
            sim_require_finite=True,
            sim_require_nnan=True,
            nc=nc,
        )
        return tuple(outs)

    devices = jax.devices()[:NCORES]
    assert len(devices) == NCORES, f"need {NCORES} cores, got {len(jax.devices())}"
    mesh = Mesh(np.asarray(devices), ("core",))
    in_specs = (PartitionSpec("core"),) * (n_params + n_outs)
    out_specs = (PartitionSpec("core"),) * n_outs
    fn = jax.jit(
        shard_map(
            _body, mesh=mesh, in_specs=in_specs, out_specs=out_specs,
            check_rep=False,
        ),
        keep_unused=True,
    )
    r = {
        "fn": fn,
        "nc": nc,
        "in_names": in_names,
        "out_names": out_names,
        "out_avals": out_avals,
        "sharding": NamedSharding(mesh, PartitionSpec("core")),
    }
    _RUNNERS[loop] = r
    return r


def _shard_inputs(x, u, v, b, W, fc_bias):
    """Build per-core device input dicts (host-side layout + bf16 cast)."""
    # xt[c][bb, p, k, f] = x[4c+bb, f, 128k+p]
    xt = np.ascontiguousarray(
        x.reshape(B, F, KT, 128).transpose(0, 3, 2, 1)
    ).astype(BF16)
    # wt[p, k, o] = W[o, 128k+p]
    wt = np.ascontiguousarray(W.reshape(OUT, KT, 128).transpose(2, 1, 0)).astype(BF16)
    # vs[bb, p, k, r] = v[bb, 0, 128k+p, r] / (IN*R)
    vs = np.ascontiguousarray(
        (v[:, 0] / float(IN * R)).reshape(B, KT, 128, R).transpose(0, 2, 1, 3)
    ).astype(BF16)
    # utb rows 0..15: u.T; row 16: fc_bias + b (the z1' ones row picks it up)
    ut = u[:, 0].transpose(0, 2, 1)  # [B, R, OUT]
    bias = fc_bias[None, None, :] + b  # [B, 1, OUT]
    utb = np.ascontiguousarray(np.concatenate([ut, bias], axis=1)).astype(BF16)

    in_maps = []
    for c in range(NCORES):
        s = slice(c * BPC, (c + 1) * BPC)
        in_maps.append({"xt": xt[s], "wt": wt, "vs": vs[s], "utb": utb[s]})
    return in_maps


def _device_args(runner, in_maps):
    """Concat per-core inputs to global arrays and place them on device.

    Returns device-resident jax.Arrays (inputs + reusable zero buffers for
    the output operands) that can be passed to runner['fn'] repeatedly with
    no further host<->device traffic.
    """
    import jax

    nc = runner["nc"]
    if getattr(nc, "dbg_addr", None) is not None:
        zdbg = np.zeros((1, 2), np.uint32)
        in_maps = [{**m, nc.dbg_addr.name: zdbg} for m in in_maps]
    concat = [
        np.concatenate([np.asarray(m[name]) for m in in_maps], axis=0)
        for name in runner["in_names"]
    ]
    zeros = [
        np.zeros((NCORES * a.shape[0], *a.shape[1:]), a.dtype)
        for a in runner["out_avals"]
    ]
    return [jax.device_put(a, runner["sharding"]) for a in concat + zeros]


def kernel(x, u, v, b, W, fc_bias):
    x = np.asarray(x, dtype=np.float32)
    u = np.asarray(u, dtype=np.float32)
    v = np.asarray(v, dtype=np.float32)
    b = np.asarray(b, dtype=np.float32)
    W = np.asarray(W, dtype=np.float32)
    fc_bias = np.asarray(fc_bias, dtype=np.float32)

    in_maps = _shard_inputs(x, u, v, b, W, fc_bias)
    runner = _get_runner(loop=1)
    dev = _device_args(runner, in_maps)
    outs = runner["fn"](*dev)
    yi = runner["out_names"].index("y")
    yb = np.asarray(outs[yi])  # [B, FT, 128, OUT] bf16 global
    return np.ascontiguousarray(yb.astype(np.float32).reshape(B, F, OUT))
